# revision 27
# baseline (speedup 1.0000x reference)
"""Trainium2 Bass kernel for CifNet conv-QKV self-attention.

Sharding: 8 cores = 4 (batch) x 2 (head-groups of 4 heads).
Each core computes, for its batch sample b and head-group g:
  - q/k/v = conv3x3(x, w{q,k,v}[g*256:(g+1)*256])   (256 out-channels = 4 heads)
  - per-head attention over hw=2304 positions (softmax without max-subtraction,
    denominator fused into the AV matmul via an appended ones-column on V^T)
  - partial o-conv: conv3x3(attn_out, wo[:, g*256:(g+1)*256])  -> [256, 2304] fp32
Host sums the two head-group partials per batch sample.

Convs are expressed as 9 shifted matmuls (one per tap) accumulating in PSUM,
with the input pre-padded to [C, 50, 50] on the host. All matmuls run in bf16
with fp32 PSUM accumulation.

Schedule notes (v2):
  - Attention q-tiles of 480 columns (10 output rows; last tile 384/8 rows),
    units of 2 kj-tiles. Per unit and kj-tile j, the two heads' score MMs
    write one 2-bank PSUM tile (slice hh) and become ready together via a
    single WAR dep on the previous use's exp ACTIVATE, so they schedule
    adjacently and run CONCURRENTLY in disjoint PE row groups
    (tile_position (0,0)/(64,0)) -- halving score matmul time.
  - Attention m0 starts after only v-rows 0-1, 12 transposes, k-rows 0-1 and
    q-rows 0-1; all remaining conv work drips between attention units
    just-in-time, so the exp ACTIVATE chain starts ~15us into the kernel.
  - o-conv kg1 row units interleave into m1 attention as their opad row
    dependencies resolve, shrinking the serial tail to 2 row units.
"""

from contextlib import ExitStack

import numpy as np
import ml_dtypes

# problem shape (hardcoded per contract)
B, C, H, W = 4, 256, 48, 48
HW = H * W              # 2304
NCORES = 8
RT = 8                  # conv output rows per spatial tile
NT = RT * W             # 384 columns per conv matmul
NROW = H // RT          # 6 conv row tiles
NKJ = HW // 128         # 18 key tiles
NUNIT = NKJ // 2        # 9 attention units per (m, qi)

# attention q-tiling: (col0, width, out_row0, n_out_rows). The first tile is
# 384 cols = exactly q-conv row 0, so attention starts before the wq DMA has
# fed q-conv row 1.
QTS = [(0, 384, 0, 8), (384, 480, 8, 10), (864, 480, 18, 10),
       (1344, 480, 28, 10), (1824, 480, 38, 10)]

_cached = None


def _build():
    """Build and compile the per-core SPMD Bass program (cached)."""
    global _cached
    if _cached is not None:
        return _cached

    import concourse.bass as bass  # noqa: F401
    import concourse.tile as tile
    from concourse import bacc, mybir
    from concourse.masks import make_identity

    BF = mybir.dt.bfloat16
    F32 = mybir.dt.float32
    EXP = mybir.ActivationFunctionType.Exp

    nc = bacc.Bacc("TRN2", target_bir_lowering=False, debug=False)
    x_d = nc.dram_tensor("xpad", [2, 128, 50, 50], BF, kind="ExternalInput").ap()
    wqkv_d = nc.dram_tensor("wqkv", [3, 9, 2, 128, 256], BF, kind="ExternalInput").ap()
    wo_d = nc.dram_tensor("wo", [9, 2, 128, 256], BF, kind="ExternalInput").ap()
    out_d = nc.dram_tensor("out", [2, 128, HW], F32, kind="ExternalOutput").ap()

    with tile.TileContext(nc) as tc, ExitStack() as ctx:
        konst = ctx.enter_context(tc.tile_pool(name="konst", bufs=1))
        ident = konst.tile([128, 64], BF, name="ident")
        x_sb = konst.tile([128, 2, 50, 50], BF, name="x_sb")
        wq_sb = konst.tile([128, 9, 2, 256], BF, name="wq_sb")
        wk_sb = konst.tile([128, 9, 2, 256], BF, name="wk_sb")
        wv_sb = konst.tile([128, 9, 2, 256], BF, name="wv_sb")
        wo_sb = konst.tile([128, 9, 2, 256], BF, name="wo_sb")
        q_sb = [konst.tile([128, HW], BF, name=f"q_sb{m}") for m in range(2)]
        k_sb = [konst.tile([128, HW], BF, name=f"k_sb{m}") for m in range(2)]
        v_sb = [konst.tile([128, HW], BF, name=f"v_sb{m}") for m in range(2)]
        # V^T per head: [kj within tile, kj tile, 65]; col 64 holds ones so the
        # AV matmul also produces the softmax denominator in psum row 64.
        vt_sb = [konst.tile([128, NKJ, 65], BF, name=f"vt_sb{h}") for h in range(4)]
        opad = [konst.tile([128, 50, 50], BF, name=f"opad{g}") for g in range(2)]
        osum = [konst.tile([128, HW], F32, name=f"osum{mo}") for mo in range(2)]

        # input DMAs FIRST on the sync queue (anything emitted earlier would
        # head-of-line block them), ordered by first use: x kg0, wv kg0, then
        # the tiny identity setup (needed by transposes ~9us in), then the
        # rest -- per-tap transfers spread across queues.
        nc.sync.dma_start(x_sb[:, 0], x_d[0])
        for t in range(9):
            nc.sync.dma_start(wv_sb[:, t, 0], wqkv_d[2, t, 0])

        # identity for PE transpose; duplicated at base partitions 0 and 64 so
        # the transpose input/identity share a base partition
        make_identity(nc, ident[0:64, :])
        nc.sync.dma_start(ident[64:128, :], ident[0:64, :])

        nc.sync.dma_start(x_sb[:, 1], x_d[1])
        for t in range(9):
            nc.sync.dma_start(wv_sb[:, t, 1], wqkv_d[2, t, 1])
        for t in range(9):
            nc.sync.dma_start(wk_sb[:, t], wqkv_d[1, t].rearrange("g p o -> p g o"))
        for t in range(9):
            nc.sync.dma_start(wq_sb[:, t], wqkv_d[0, t].rearrange("g p o -> p g o"))
        for t in range(9):
            nc.sync.dma_start(wo_sb[:, t], wo_d[t].rearrange("g p o -> p g o"))

        for h in range(4):
            nc.gpsimd.memset(vt_sb[h][:], 1.0)
        for g in range(2):
            nc.gpsimd.memset(opad[g][:], 0.0)

        # warm the ACT exp table during the DMA phase (one-time ~2.7us load)
        wrm = konst.tile([1, 8], F32, name="wrm")
        nc.gpsimd.memset(wrm[:], 0.0)
        nc.scalar.activation(wrm[:], wrm[:], EXP, scale=0.125)

        with tc.tile_pool(name="spsum", bufs=2, space="PSUM") as spsum, \
             tc.tile_pool(name="apsum", bufs=2, space="PSUM") as apsum, \
             tc.tile_pool(name="fpsum", bufs=2, space="PSUM") as fpsum, \
             tc.tile_pool(name="esb", bufs=4) as esb, \
             tc.tile_pool(name="osb", bufs=3) as osb, \
             tc.tile_pool(name="nsb", bufs=2) as nsb:

            def conv_row_unit(m, w_sb, dst, r):
                """One rowtile of a conv: 18 accumulating MMs into 1 psum bank."""
                ps = fpsum.tile([128, NT], F32, tag="fps", name="fps")
                first = True
                for kg in range(2):
                    for t in range(9):
                        ky, kx = t // 3, t % 3
                        lhsT = w_sb[:, t, kg, m * 128:(m + 1) * 128]
                        rhs = x_sb[:, kg, r * RT + ky: r * RT + ky + RT, kx: kx + W]
                        nc.tensor.matmul(ps[:], lhsT, rhs, start=first,
                                         stop=(kg == 1 and t == 8))
                        first = False
                nc.vector.tensor_copy(dst[:, r * NT:(r + 1) * NT], ps[:])

            def transpose_unit(m, hh, kt):
                h = 2 * m + hh
                pt = fpsum.tile([128, 64], BF, tag="fps", name="tps")
                nc.tensor.transpose(
                    pt[:],
                    v_sb[m][64 * hh:64 * hh + 64, kt * 128:(kt + 1) * 128],
                    ident[64 * hh:64 * hh + 64, :],
                )
                nc.vector.tensor_copy(vt_sb[h][:, kt, 0:64], pt[:])

            def oconv_row_unit(mo, r, kg):
                """One rowtile of the o-conv for one input kgroup (9 taps)."""
                ps = fpsum.tile([128, NT], F32, tag="fps", name="fps")
                for t in range(9):
                    ky, kx = t // 3, t % 3
                    lhsT = wo_sb[:, t, kg, mo * 128:(mo + 1) * 128]
                    rhs = opad[kg][:, r * RT + ky: r * RT + ky + RT, kx: kx + W]
                    nc.tensor.matmul(ps[:], lhsT, rhs, start=(t == 0), stop=(t == 8))
                if kg == 0:
                    nc.vector.tensor_copy(osum[mo][:, r * NT:(r + 1) * NT], ps[:])
                else:
                    ot = osb.tile([128, NT], F32, tag="osb", name="osb")
                    nc.vector.tensor_tensor(
                        ot[:], ps[:], osum[mo][:, r * NT:(r + 1) * NT],
                        mybir.AluOpType.add,
                    )
                    nc.sync.dma_start(out_d[mo, :, r * NT:(r + 1) * NT], ot[:])

            def att_unit(m, q0, qw, u):
                """One attention unit: 2 kj tiles x 2 heads.

                Per kj tile: both heads' score MMs write one 2-bank psum tile
                (slice hh) so they become ready together -> adjacent in the
                schedule -> concurrent in disjoint PE row groups. One exp
                ACTIVATE per kj tile covers both heads (960 cols).
                """
                for j in range(2):
                    kjt = 2 * u + j
                    sp = spsum.tile([128, 2, 512], F32, tag="sps", name="sps")
                    for hh in range(2):
                        nc.tensor.matmul(
                            sp[:, hh, 0:qw],
                            k_sb[m][64 * hh:64 * hh + 64, kjt * 128:(kjt + 1) * 128],
                            q_sb[m][64 * hh:64 * hh + 64, q0:q0 + qw],
                            start=True, stop=True,
                            tile_position=(64 * hh, 0),
                        )
                    et = esb.tile([128, 2, 480], BF, tag="et", name="et")
                    nc.scalar.activation(et[:, :, 0:qw], sp[:, :, 0:qw], EXP,
                                         scale=0.125)
                    for hh in range(2):
                        h = 2 * m + hh
                        nc.tensor.matmul(
                            av_cur[hh][0:65, 0:qw], vt_sb[h][:, kjt, 0:65],
                            et[:, hh, 0:qw],
                            start=(kjt == 0), stop=(kjt == NKJ - 1),
                        )

            def normalize(m, hh, q0, qw, row0, nrows):
                avf = nsb.tile([128, 480], F32, tag="avf", name="avf")
                nc.vector.tensor_copy(avf[0:65, 0:qw], av_cur[hh][0:65, 0:qw])
                dn = nsb.tile([1, 480], F32, tag="dn", name="dn")
                nc.sync.dma_start(dn[0:1, 0:qw], avf[64:65, 0:qw])
                rc = nsb.tile([1, 480], F32, tag="rc", name="rc")
                nc.vector.reciprocal_approx_fast(rc[0:1, 0:qw], dn[0:1, 0:qw])
                rb = nsb.tile([64, 480], F32, tag="rb", name="rb")
                nc.gpsimd.partition_broadcast(rb[:, 0:qw], rc[0:1, 0:qw])
                tmp = nsb.tile([64, 480], BF, tag="tmp", name="tmp")
                nc.vector.tensor_mul(tmp[:, 0:qw], avf[0:64, 0:qw], rb[:, 0:qw])
                dst = opad[m][64 * hh:64 * hh + 64, row0 + 1: row0 + nrows + 1, 1:49]
                nc.sync.dma_start(
                    dst, tmp[:, 0:qw].rearrange("p (r c) -> p r c", c=W))

            def run_attention(m, sched, at_qi_end=None):
                """Emit attention for head-pair m; sched[(qi, u)] = fillers."""
                for qi, (q0, qw, row0, nrows) in enumerate(QTS):
                    av_cur[0] = apsum.tile([128, 512], F32, tag="avps", name="avps")
                    av_cur[1] = apsum.tile([128, 512], F32, tag="avps", name="avps")
                    for u in range(NUNIT):
                        att_unit(m, q0, qw, u)
                        for fn in sched.get((qi, u), ()):
                            fn()
                    for hh in range(2):
                        normalize(m, hh, q0, qw, row0, nrows)
                    if at_qi_end is not None:
                        at_qi_end(qi)

            av_cur = [None, None]

            def T(m, kts):
                return [lambda hh=hh, kt=kt, m=m: transpose_unit(m, hh, kt)
                        for kt in kts for hh in range(2)]

            def CR(m, w, dst, rows):
                return [lambda r=r, m=m, w=w, dst=dst: conv_row_unit(m, w, dst, r)
                        for r in rows]

            # ---- phase A prefix: minimum work before attention m0 can start.
            # v rows 0-3 first (their weights arrive earliest), then k/q rows
            # 0-1 (wk/wq DMAs land ~11/18us in; v-conv keeps the PE warm).
            for fn in (CR(0, wv_sb, v_sb[0], [0, 1]) + T(0, [0, 1, 2, 3, 4, 5])
                       + CR(0, wv_sb, v_sb[0], [2, 3]) + T(0, [6, 7, 8, 9, 10, 11])
                       + CR(0, wk_sb, k_sb[0], [0, 1])
                       + CR(0, wq_sb, q_sb[0], [0])):
                fn()

            # ---- phase B: attention m0; fillers drip just-in-time.
            # (unit u consumes kj tiles 2u,2u+1; conv row r covers kj 3r..3r+2;
            # qi tile i needs q-conv rows covering cols 480i..480i+479)
            # m1 conv work is split between late B and early C so that both
            # phases stay PE-bound (a phase with idle PE goes HAM-cold and
            # then pays ~2x per matmul).
            sched_b = {
                (0, 0): CR(0, wq_sb, q_sb[0], [1]),
                (0, 1): CR(0, wk_sb, k_sb[0], [2]),
                (0, 2): CR(0, wv_sb, v_sb[0], [4]) + T(0, [12, 13]),
                (0, 3): CR(0, wk_sb, k_sb[0], [3]),
                (0, 4): CR(0, wv_sb, v_sb[0], [5]) + T(0, [14, 15]),
                (0, 5): CR(0, wk_sb, k_sb[0], [4]) + T(0, [16, 17]),
                (0, 6): CR(0, wk_sb, k_sb[0], [5]),
                (0, 7): CR(0, wq_sb, q_sb[0], [2]),
                (1, 0): CR(0, wq_sb, q_sb[0], [3]),
                (1, 4): CR(1, wv_sb, v_sb[1], [0]) + T(1, [0, 1, 2]),
                (2, 0): CR(0, wq_sb, q_sb[0], [4]),
                (2, 4): CR(1, wv_sb, v_sb[1], [1]) + T(1, [3, 4, 5]),
                (3, 0): CR(0, wq_sb, q_sb[0], [5]),
                (3, 2): CR(1, wk_sb, k_sb[1], [0]),
                (3, 4): CR(1, wv_sb, v_sb[1], [2]) + T(1, [6, 7, 8]),
                (4, 0): CR(1, wk_sb, k_sb[1], [1]),
                (4, 2): CR(1, wv_sb, v_sb[1], [3]) + T(1, [9, 10, 11]),
                (4, 4): CR(1, wq_sb, q_sb[1], [0]),
                (4, 6): CR(1, wq_sb, q_sb[1], [1]),
            }
            run_attention(0, sched_b)

            # ---- phase C: attention m1; fillers = remaining m1 conv rows
            # (just-in-time) + o-conv kg0 rows; kg1 rows interleave at qi
            # boundaries as their opad[1] row dependencies resolve.
            OC = lambda mo, r, kg: (lambda: oconv_row_unit(mo, r, kg))
            sched_c = {
                (0, 0): CR(1, wk_sb, k_sb[1], [2]),
                (0, 1): CR(1, wv_sb, v_sb[1], [4]) + T(1, [12, 13, 14]),
                (0, 2): CR(1, wk_sb, k_sb[1], [3]),
                (0, 3): CR(1, wv_sb, v_sb[1], [5]) + T(1, [15, 16, 17]),
                (0, 4): CR(1, wk_sb, k_sb[1], [4]),
                (0, 5): CR(1, wk_sb, k_sb[1], [5]),
                (0, 6): CR(1, wq_sb, q_sb[1], [2]),
                (0, 7): [OC(0, 0, 0)],
                (0, 8): [OC(1, 0, 0)],
                (1, 0): CR(1, wq_sb, q_sb[1], [3]),
                (1, 3): [OC(0, 1, 0)],
                (1, 6): [OC(1, 1, 0)],
                (1, 8): [OC(0, 2, 0)],
                (2, 0): CR(1, wq_sb, q_sb[1], [4]),
                (2, 3): [OC(1, 2, 0)],
                (2, 6): [OC(0, 3, 0)],
                (2, 8): [OC(1, 3, 0)],
                (3, 0): CR(1, wq_sb, q_sb[1], [5]),
                (3, 4): [OC(0, 4, 0)],
                (3, 8): [OC(1, 4, 0)],
            }

            def c_qi_end(qi):
                # kg1 row r reads opad[1] unpadded rows 8r-1..8r+8;
                # qi writes rows 8/10qi.. per QTS (qi0: 0-7, qi4: 38-47).
                # kg0 row 5 is held back to here: it keeps the PE warm through
                # the final normalize chain instead of a HAM-cooling idle gap.
                if qi == 4:
                    for mo in range(2):
                        oconv_row_unit(mo, 5, 0)
                for r in {0: [], 1: [0, 1], 2: [2], 3: [3], 4: [4, 5]}[qi]:
                    for mo in range(2):
                        oconv_row_unit(mo, r, 1)

            run_attention(1, sched_c, at_qi_end=c_qi_end)

    nc.compile()
    _cached = nc
    return nc


def make_in_maps(hidden_states, wq, wk, wv, wo):
    """Shard + pre-transform full inputs into 8 per-core input dicts."""
    bf = ml_dtypes.bfloat16
    hidden_states = np.asarray(hidden_states, np.float32)
    in_maps = []
    for core in range(NCORES):
        b, g = core // 2, core % 2
        xp = np.zeros((C, 50, 50), np.float32)
        xp[:, 1:49, 1:49] = hidden_states[b]
        xpad = np.ascontiguousarray(xp.reshape(2, 128, 50, 50)).astype(bf)
        wstk = np.stack(
            [
                np.asarray(w, np.float32)[g * 256:(g + 1) * 256]
                .transpose(2, 3, 1, 0)
                .reshape(9, 2, 128, 256)
                for w in (wq, wk, wv)
            ]
        ).astype(bf)
        wog = (
            np.asarray(wo, np.float32)[:, g * 256:(g + 1) * 256]
            .transpose(2, 3, 1, 0)
            .reshape(9, 2, 128, 256)
            .astype(bf)
        )
        in_maps.append({"xpad": xpad, "wqkv": wstk, "wo": wog})
    return in_maps


def combine_outputs(per_core_outs):
    """Sum the two head-group partials per batch sample."""
    out = np.empty((B, C, H, W), np.float32)
    for b in range(B):
        acc = per_core_outs[2 * b].reshape(C, HW).astype(np.float32) + \
              per_core_outs[2 * b + 1].reshape(C, HW).astype(np.float32)
        out[b] = acc.reshape(C, H, W)
    return out


def kernel(hidden_states, wq, wk, wv, wo):
    from concourse.bass_utils import run_bass_kernel_spmd

    nc = _build()
    in_maps = make_in_maps(hidden_states, wq, wk, wv, wo)
    res = run_bass_kernel_spmd(nc, in_maps, core_ids=list(range(NCORES)))
    return combine_outputs([r["out"] for r in res.results])


# revision 31
# speedup vs baseline: 1.0035x; 1.0035x over previous
"""Trainium2 Bass kernel for CifNet conv-QKV self-attention.

Sharding: 8 cores = 4 (batch) x 2 (head-groups of 4 heads).
Each core computes, for its batch sample b and head-group g:
  - q/k/v = conv3x3(x, w{q,k,v}[g*256:(g+1)*256])   (256 out-channels = 4 heads)
  - per-head attention over hw=2304 positions (softmax without max-subtraction,
    denominator fused into the AV matmul via an appended ones-column on V^T)
  - partial o-conv: conv3x3(attn_out, wo[:, g*256:(g+1)*256])  -> [256, 2304] fp32
Host sums the two head-group partials per batch sample.

Convs are expressed as 9 shifted matmuls (one per tap) accumulating in PSUM,
with the input pre-padded to [C, 50, 50] on the host. All matmuls run in bf16
with fp32 PSUM accumulation.

Schedule notes (v2):
  - Attention q-tiles of 480 columns (10 output rows; last tile 384/8 rows),
    units of 2 kj-tiles. Per unit and kj-tile j, the two heads' score MMs
    write one 2-bank PSUM tile (slice hh) and become ready together via a
    single WAR dep on the previous use's exp ACTIVATE, so they schedule
    adjacently and run CONCURRENTLY in disjoint PE row groups
    (tile_position (0,0)/(64,0)) -- halving score matmul time.
  - Attention m0 starts after only v-rows 0-1, 12 transposes, k-rows 0-1 and
    q-rows 0-1; all remaining conv work drips between attention units
    just-in-time, so the exp ACTIVATE chain starts ~15us into the kernel.
  - o-conv kg1 row units interleave into m1 attention as their opad row
    dependencies resolve, shrinking the serial tail to 2 row units.
"""

from contextlib import ExitStack

import numpy as np
import ml_dtypes

# problem shape (hardcoded per contract)
B, C, H, W = 4, 256, 48, 48
HW = H * W              # 2304
NCORES = 8
RT = 8                  # conv output rows per spatial tile
NT = RT * W             # 384 columns per conv matmul
NROW = H // RT          # 6 conv row tiles
NKJ = HW // 128         # 18 key tiles
NUNIT = NKJ // 2        # 9 attention units per (m, qi)

# attention q-tiling: (col0, width, out_row0, n_out_rows)
QTS = [(0, 480, 0, 10), (480, 480, 10, 10), (960, 480, 20, 10),
       (1440, 480, 30, 10), (1920, 384, 40, 8)]

_cached = None


def _build():
    """Build and compile the per-core SPMD Bass program (cached)."""
    global _cached
    if _cached is not None:
        return _cached

    import concourse.bass as bass  # noqa: F401
    import concourse.tile as tile
    from concourse import bacc, mybir
    from concourse.masks import make_identity

    BF = mybir.dt.bfloat16
    F32 = mybir.dt.float32
    EXP = mybir.ActivationFunctionType.Exp

    nc = bacc.Bacc("TRN2", target_bir_lowering=False, debug=False)
    x_d = nc.dram_tensor("xpad", [2, 128, 50, 50], BF, kind="ExternalInput").ap()
    wqkv_d = nc.dram_tensor("wqkv", [3, 9, 2, 128, 256], BF, kind="ExternalInput").ap()
    wo_d = nc.dram_tensor("wo", [9, 2, 128, 256], BF, kind="ExternalInput").ap()
    out_d = nc.dram_tensor("out", [2, 128, HW], F32, kind="ExternalOutput").ap()

    with tile.TileContext(nc) as tc, ExitStack() as ctx:
        konst = ctx.enter_context(tc.tile_pool(name="konst", bufs=1))
        ident = konst.tile([128, 64], BF, name="ident")
        x_sb = konst.tile([128, 2, 50, 50], BF, name="x_sb")
        wq_sb = konst.tile([128, 9, 2, 256], BF, name="wq_sb")
        wk_sb = konst.tile([128, 9, 2, 256], BF, name="wk_sb")
        wv_sb = konst.tile([128, 9, 2, 256], BF, name="wv_sb")
        wo_sb = konst.tile([128, 9, 2, 256], BF, name="wo_sb")
        q_sb = [konst.tile([128, HW], BF, name=f"q_sb{m}") for m in range(2)]
        k_sb = [konst.tile([128, HW], BF, name=f"k_sb{m}") for m in range(2)]
        v_sb = [konst.tile([128, HW], BF, name=f"v_sb{m}") for m in range(2)]
        # V^T per head: [kj within tile, kj tile, 65]; col 64 holds ones so the
        # AV matmul also produces the softmax denominator in psum row 64.
        vt_sb = [konst.tile([128, NKJ, 65], BF, name=f"vt_sb{h}") for h in range(4)]
        opad = [konst.tile([128, 50, 50], BF, name=f"opad{g}") for g in range(2)]
        osum = [konst.tile([128, HW], F32, name=f"osum{mo}") for mo in range(2)]

        # input DMAs FIRST on the sync queue (anything emitted earlier would
        # head-of-line block them), ordered by first use: x kg0, wv kg0, then
        # the tiny identity setup (needed by transposes ~9us in), then the
        # rest -- per-tap transfers spread across queues.
        nc.sync.dma_start(x_sb[:, 0], x_d[0])
        for t in range(9):
            nc.sync.dma_start(wv_sb[:, t, 0], wqkv_d[2, t, 0])

        # identity for PE transpose; duplicated at base partitions 0 and 64 so
        # the transpose input/identity share a base partition
        make_identity(nc, ident[0:64, :])
        nc.sync.dma_start(ident[64:128, :], ident[0:64, :])

        nc.sync.dma_start(x_sb[:, 1], x_d[1])
        for t in range(9):
            nc.sync.dma_start(wv_sb[:, t, 1], wqkv_d[2, t, 1])
        for t in range(9):
            nc.sync.dma_start(wk_sb[:, t], wqkv_d[1, t].rearrange("g p o -> p g o"))
        for t in range(9):
            nc.sync.dma_start(wq_sb[:, t], wqkv_d[0, t].rearrange("g p o -> p g o"))
        for t in range(9):
            nc.sync.dma_start(wo_sb[:, t], wo_d[t].rearrange("g p o -> p g o"))

        for h in range(4):
            nc.gpsimd.memset(vt_sb[h][:], 1.0)
        for g in range(2):
            nc.gpsimd.memset(opad[g][:], 0.0)

        # warm the ACT exp table during the DMA phase (one-time ~2.7us load)
        wrm = konst.tile([1, 8], F32, name="wrm")
        nc.gpsimd.memset(wrm[:], 0.0)
        nc.scalar.activation(wrm[:], wrm[:], EXP, scale=0.125)

        with tc.tile_pool(name="spsum", bufs=2, space="PSUM") as spsum, \
             tc.tile_pool(name="apsum", bufs=2, space="PSUM") as apsum, \
             tc.tile_pool(name="fpsum", bufs=2, space="PSUM") as fpsum, \
             tc.tile_pool(name="esb", bufs=4) as esb, \
             tc.tile_pool(name="osb", bufs=3) as osb, \
             tc.tile_pool(name="nsb", bufs=2) as nsb:

            def conv_row_unit(m, w_sb, dst, r):
                """One rowtile of a conv: 18 accumulating MMs into 1 psum bank."""
                ps = fpsum.tile([128, NT], F32, tag="fps", name="fps")
                first = True
                for kg in range(2):
                    for t in range(9):
                        ky, kx = t // 3, t % 3
                        lhsT = w_sb[:, t, kg, m * 128:(m + 1) * 128]
                        rhs = x_sb[:, kg, r * RT + ky: r * RT + ky + RT, kx: kx + W]
                        nc.tensor.matmul(ps[:], lhsT, rhs, start=first,
                                         stop=(kg == 1 and t == 8))
                        first = False
                nc.vector.tensor_copy(dst[:, r * NT:(r + 1) * NT], ps[:])

            def transpose_unit(m, hh, kt):
                h = 2 * m + hh
                pt = fpsum.tile([128, 64], BF, tag="fps", name="tps")
                nc.tensor.transpose(
                    pt[:],
                    v_sb[m][64 * hh:64 * hh + 64, kt * 128:(kt + 1) * 128],
                    ident[64 * hh:64 * hh + 64, :],
                )
                nc.vector.tensor_copy(vt_sb[h][:, kt, 0:64], pt[:])

            def oconv_row_unit(mo, r, kg):
                """One rowtile of the o-conv for one input kgroup (9 taps)."""
                ps = fpsum.tile([128, NT], F32, tag="fps", name="fps")
                for t in range(9):
                    ky, kx = t // 3, t % 3
                    lhsT = wo_sb[:, t, kg, mo * 128:(mo + 1) * 128]
                    rhs = opad[kg][:, r * RT + ky: r * RT + ky + RT, kx: kx + W]
                    nc.tensor.matmul(ps[:], lhsT, rhs, start=(t == 0), stop=(t == 8))
                if kg == 0:
                    nc.vector.tensor_copy(osum[mo][:, r * NT:(r + 1) * NT], ps[:])
                else:
                    ot = osb.tile([128, NT], F32, tag="osb", name="osb")
                    nc.vector.tensor_tensor(
                        ot[:], ps[:], osum[mo][:, r * NT:(r + 1) * NT],
                        mybir.AluOpType.add,
                    )
                    nc.sync.dma_start(out_d[mo, :, r * NT:(r + 1) * NT], ot[:])

            def att_unit(m, q0, qw, u):
                """One attention unit: 2 kj tiles x 2 heads.

                Per kj tile: both heads' score MMs write one 2-bank psum tile
                (slice hh) so they become ready together -> adjacent in the
                schedule -> concurrent in disjoint PE row groups. One exp
                ACTIVATE per kj tile covers both heads (960 cols).
                """
                for j in range(2):
                    kjt = 2 * u + j
                    sp = spsum.tile([128, 2, 512], F32, tag="sps", name="sps")
                    for hh in range(2):
                        nc.tensor.matmul(
                            sp[:, hh, 0:qw],
                            k_sb[m][64 * hh:64 * hh + 64, kjt * 128:(kjt + 1) * 128],
                            q_sb[m][64 * hh:64 * hh + 64, q0:q0 + qw],
                            start=True, stop=True,
                            tile_position=(64 * hh, 0),
                        )
                    et = esb.tile([128, 2, 480], BF, tag="et", name="et")
                    nc.scalar.activation(et[:, :, 0:qw], sp[:, :, 0:qw], EXP,
                                         scale=0.125)
                    for hh in range(2):
                        h = 2 * m + hh
                        nc.tensor.matmul(
                            av_cur[hh][0:65, 0:qw], vt_sb[h][:, kjt, 0:65],
                            et[:, hh, 0:qw],
                            start=(kjt == 0), stop=(kjt == NKJ - 1),
                        )

            def normalize(m, hh, q0, qw, row0, nrows):
                avf = nsb.tile([128, 480], F32, tag="avf", name="avf")
                nc.vector.tensor_copy(avf[0:65, 0:qw], av_cur[hh][0:65, 0:qw])
                dn = nsb.tile([1, 480], F32, tag="dn", name="dn")
                nc.sync.dma_start(dn[0:1, 0:qw], avf[64:65, 0:qw])
                rc = nsb.tile([1, 480], F32, tag="rc", name="rc")
                nc.vector.reciprocal_approx_fast(rc[0:1, 0:qw], dn[0:1, 0:qw])
                rb = nsb.tile([64, 480], F32, tag="rb", name="rb")
                nc.gpsimd.partition_broadcast(rb[:, 0:qw], rc[0:1, 0:qw])
                tmp = nsb.tile([64, 480], BF, tag="tmp", name="tmp")
                nc.vector.tensor_mul(tmp[:, 0:qw], avf[0:64, 0:qw], rb[:, 0:qw])
                dst = opad[m][64 * hh:64 * hh + 64, row0 + 1: row0 + nrows + 1, 1:49]
                nc.sync.dma_start(
                    dst, tmp[:, 0:qw].rearrange("p (r c) -> p r c", c=W))

            def run_attention(m, sched, at_qi_end=None):
                """Emit attention for head-pair m; sched[(qi, u)] = fillers."""
                for qi, (q0, qw, row0, nrows) in enumerate(QTS):
                    av_cur[0] = apsum.tile([128, 512], F32, tag="avps", name="avps")
                    av_cur[1] = apsum.tile([128, 512], F32, tag="avps", name="avps")
                    for u in range(NUNIT):
                        att_unit(m, q0, qw, u)
                        for fn in sched.get((qi, u), ()):
                            fn()
                    for hh in range(2):
                        normalize(m, hh, q0, qw, row0, nrows)
                    if at_qi_end is not None:
                        at_qi_end(qi)

            av_cur = [None, None]

            def T(m, kts):
                return [lambda hh=hh, kt=kt, m=m: transpose_unit(m, hh, kt)
                        for kt in kts for hh in range(2)]

            def CR(m, w, dst, rows):
                return [lambda r=r, m=m, w=w, dst=dst: conv_row_unit(m, w, dst, r)
                        for r in rows]

            # ---- phase A prefix: minimum work before attention m0 can start.
            # v rows 0-3 first (their weights arrive earliest), then k/q rows
            # 0-1 (wk/wq DMAs land ~11/18us in; v-conv keeps the PE warm).
            for fn in (CR(0, wv_sb, v_sb[0], [0, 1]) + T(0, [0, 1, 2, 3, 4, 5])
                       + CR(0, wv_sb, v_sb[0], [2, 3]) + T(0, [6, 7, 8, 9, 10, 11])
                       + CR(0, wk_sb, k_sb[0], [0, 1])
                       + CR(0, wq_sb, q_sb[0], [0, 1])):
                fn()

            # ---- phase B: attention m0; fillers drip just-in-time.
            # (unit u consumes kj tiles 2u,2u+1; conv row r covers kj 3r..3r+2;
            # qi tile i needs q-conv rows covering cols 480i..480i+479)
            # m1 conv work is split between late B and early C so that both
            # phases stay PE-bound (a phase with idle PE goes HAM-cold and
            # then pays ~2x per matmul).
            sched_b = {
                (0, 0): CR(0, wk_sb, k_sb[0], [2]),
                (0, 1): CR(0, wv_sb, v_sb[0], [4]) + T(0, [12, 13]),
                (0, 2): CR(0, wk_sb, k_sb[0], [3]),
                (0, 3): CR(0, wv_sb, v_sb[0], [5]) + T(0, [14, 15]),
                (0, 4): CR(0, wk_sb, k_sb[0], [4]) + T(0, [16, 17]),
                (0, 5): CR(0, wk_sb, k_sb[0], [5]),
                (0, 6): CR(0, wq_sb, q_sb[0], [2]),
                (1, 0): CR(0, wq_sb, q_sb[0], [3]),
                (1, 4): CR(1, wv_sb, v_sb[1], [0]) + T(1, [0, 1, 2]),
                (2, 0): CR(0, wq_sb, q_sb[0], [4]),
                (2, 4): CR(1, wv_sb, v_sb[1], [1]) + T(1, [3, 4, 5]),
                (3, 0): CR(0, wq_sb, q_sb[0], [5]),
                (3, 2): CR(1, wk_sb, k_sb[1], [0]),
                (3, 4): CR(1, wv_sb, v_sb[1], [2]) + T(1, [6, 7, 8]),
                (4, 0): CR(1, wk_sb, k_sb[1], [1]),
                (4, 2): CR(1, wv_sb, v_sb[1], [3]) + T(1, [9, 10, 11]),
                (4, 4): CR(1, wq_sb, q_sb[1], [0]),
                (4, 6): CR(1, wq_sb, q_sb[1], [1]),
            }
            run_attention(0, sched_b)

            # ---- phase C: attention m1; fillers = remaining m1 conv rows
            # (just-in-time) + o-conv kg0 rows; kg1 rows interleave at qi
            # boundaries as their opad[1] row dependencies resolve.
            OC = lambda mo, r, kg: (lambda: oconv_row_unit(mo, r, kg))
            sched_c = {
                (0, 0): CR(1, wk_sb, k_sb[1], [2]),
                (0, 1): CR(1, wv_sb, v_sb[1], [4]) + T(1, [12, 13, 14]),
                (0, 2): CR(1, wk_sb, k_sb[1], [3]),
                (0, 3): CR(1, wv_sb, v_sb[1], [5]) + T(1, [15, 16, 17]),
                (0, 4): CR(1, wk_sb, k_sb[1], [4]),
                (0, 5): CR(1, wk_sb, k_sb[1], [5]),
                (0, 6): CR(1, wq_sb, q_sb[1], [2]),
                (0, 7): [OC(0, 0, 0)],
                (0, 8): [OC(1, 0, 0)],
                (1, 0): CR(1, wq_sb, q_sb[1], [3]),
                (1, 3): [OC(0, 1, 0)],
                (1, 6): [OC(1, 1, 0)],
                (1, 8): [OC(0, 2, 0)],
                (2, 0): CR(1, wq_sb, q_sb[1], [4]),
                (2, 3): [OC(1, 2, 0)],
                (2, 6): [OC(0, 3, 0)],
                (2, 8): [OC(1, 3, 0)],
                (3, 0): CR(1, wq_sb, q_sb[1], [5]),
                (3, 4): [OC(0, 4, 0)],
                (3, 8): [OC(1, 4, 0)],
            }

            def c_qi_end(qi):
                # kg1 row r reads opad[1] unpadded rows 8r-1..8r+8;
                # qi writes rows 8/10qi.. per QTS (qi0: 0-7, qi4: 38-47).
                # kg0 row 5 is held back to here: it keeps the PE warm through
                # the final normalize chain instead of a HAM-cooling idle gap.
                if qi == 4:
                    for mo in range(2):
                        oconv_row_unit(mo, 5, 0)
                for r in {0: [0], 1: [1], 2: [2], 3: [3], 4: [4, 5]}[qi]:
                    for mo in range(2):
                        oconv_row_unit(mo, r, 1)

            run_attention(1, sched_c, at_qi_end=c_qi_end)

    nc.compile()
    _cached = nc
    return nc


def make_in_maps(hidden_states, wq, wk, wv, wo):
    """Shard + pre-transform full inputs into 8 per-core input dicts."""
    bf = ml_dtypes.bfloat16
    hidden_states = np.asarray(hidden_states, np.float32)
    in_maps = []
    for core in range(NCORES):
        b, g = core // 2, core % 2
        xp = np.zeros((C, 50, 50), np.float32)
        xp[:, 1:49, 1:49] = hidden_states[b]
        xpad = np.ascontiguousarray(xp.reshape(2, 128, 50, 50)).astype(bf)
        wstk = np.stack(
            [
                np.asarray(w, np.float32)[g * 256:(g + 1) * 256]
                .transpose(2, 3, 1, 0)
                .reshape(9, 2, 128, 256)
                for w in (wq, wk, wv)
            ]
        ).astype(bf)
        wog = (
            np.asarray(wo, np.float32)[:, g * 256:(g + 1) * 256]
            .transpose(2, 3, 1, 0)
            .reshape(9, 2, 128, 256)
            .astype(bf)
        )
        in_maps.append({"xpad": xpad, "wqkv": wstk, "wo": wog})
    return in_maps


def combine_outputs(per_core_outs):
    """Sum the two head-group partials per batch sample."""
    out = np.empty((B, C, H, W), np.float32)
    for b in range(B):
        acc = per_core_outs[2 * b].reshape(C, HW).astype(np.float32) + \
              per_core_outs[2 * b + 1].reshape(C, HW).astype(np.float32)
        out[b] = acc.reshape(C, H, W)
    return out


def kernel(hidden_states, wq, wk, wv, wo):
    from concourse.bass_utils import run_bass_kernel_spmd

    nc = _build()
    in_maps = make_in_maps(hidden_states, wq, wk, wv, wo)
    res = run_bass_kernel_spmd(nc, in_maps, core_ids=list(range(NCORES)))
    return combine_outputs([r["out"] for r in res.results])
